# revision 14
# baseline (speedup 1.0000x reference)
"""AttnDecoder single-step kernel for Trainium2, 8-way SPMD.

Strategy (vocab/tensor parallel):
  - GRU gate rows (r,z,n each H=1024) sharded 8 ways -> each core computes a
    128-row slice of h_new via DVE fused mul-reduce matvecs; AllGather #1
    (4KB) -> full h_new everywhere.
  - Attention: encoder rows (S=8192) sharded 8 ways; local dot-scores via DVE
    fused mul-reduce; local softmax stats + unnormalized context partial (PE);
    AllGather #2 (~4KB/rank packet: [context_partial, m_k, z_k]) -> global
    softmax combine on every core.
  - Output projection: vocab dim padded to 50304 = 8*6288, column-sharded;
    W_out pre-transposed on host to [2H, Vk] fp16 and split into an h_new-half
    and a context-half so the h_new half streams through the PE while the
    attention stage is still running.  fp16 matmul, fp32 PSUM accumulate,
    per-chunk max/exp-sum stats; AllGather #3 (8B/rank) -> global logsumexp;
    subtract, write log-softmax shard.
  - Stage-A tiles (GRU weights, x broadcast, encoder shard) share one tile
    pool tag with the W_out stream tiles so their SBUF recycles into the ring.

Host does: embedding row gather (1 row of the [V,H] table), input layout prep
(shard + transpose), and output concatenation.
"""

import numpy as np

import concourse.bass as bass
import concourse.bacc as bacc
import concourse.tile as tile
from concourse import mybir
from concourse.bass_utils import run_bass_kernel_spmd

NCORES = 8
H = 1024
S = 8192
V = 50257
HK = H // NCORES          # 128   h-slice per core
SK = S // NCORES          # 1024  encoder rows per core
VK = 6288                 # vocab columns per core (8*6288 = 50304 >= V)
VP = VK * NCORES          # padded vocab
VKP = 6656                # 13*512, bias/out padding for chunk layout
NCH = VKP // 512          # 13 chunks of 512
KC = 16                   # 2H / 128 contraction chunks
F32 = mybir.dt.float32
F16 = mybir.dt.float16
AX = mybir.AxisListType.X
OP = mybir.AluOpType
AF = mybir.ActivationFunctionType
NEG = -1.0e30

# W_out v-stripes; each (stripe, kc-group-of-4) is one DMA.  Stripe chunk
# count is capped at 4 so live PSUM waves fit the pool.
V_STRIPES = [(0, 2048), (2048, 2048), (4096, 1024), (5120, 1168)]

_CACHE = {}


def _build(stage="full"):
    nc = bacc.Bacc(trn_type="TRN2", num_devices=NCORES)
    rg = [list(range(NCORES))]

    # ---- per-core external inputs (host-prepped layouts) ----
    t_wih = nc.dram_tensor("wih", [128, 3, 2 * H], F32, kind="ExternalInput")
    t_whh = nc.dram_tensor("whh", [128, 3, H], F32, kind="ExternalInput")
    t_xg = nc.dram_tensor("xg", [2 * H], F32, kind="ExternalInput")
    t_hvec = nc.dram_tensor("hvec", [H], F32, kind="ExternalInput")
    t_hown = nc.dram_tensor("hown", [HK], F32, kind="ExternalInput")
    t_bg = nc.dram_tensor("bg", [128, 4], F32, kind="ExternalInput")
    t_enc = nc.dram_tensor("enc", [128, 8, H], F32, kind="ExternalInput")
    t_wout = nc.dram_tensor("wout", [KC, 128, VK], F16, kind="ExternalInput")
    t_bout = nc.dram_tensor("bout", [VKP], F32, kind="ExternalInput")

    # ---- per-core external outputs ----
    t_out = nc.dram_tensor("out_ls", [VKP], F32, kind="ExternalOutput")
    t_hout = nc.dram_tensor("h_out", [H], F32, kind="ExternalOutput")
    t_cout = nc.dram_tensor("ctx_out", [H], F32, kind="ExternalOutput")
    t_aout = nc.dram_tensor("attn_out", [SK], F32, kind="ExternalOutput")

    from contextlib import ExitStack

    class _StageCutT(Exception):
        pass
    _StageCut = _StageCutT()

    with tile.TileContext(nc) as tc, ExitStack() as ctx:
      try:
        consts = ctx.enter_context(tc.tile_pool(name="consts", bufs=1))
        big = ctx.enter_context(tc.tile_pool(name="big", bufs=4))
        psA = ctx.enter_context(tc.tile_pool(name="psA", bufs=1, space="PSUM"))
        psB = ctx.enter_context(tc.tile_pool(name="psB", bufs=4, space="PSUM"))
        dram = ctx.enter_context(tc.tile_pool(name="dram", bufs=1, space="DRAM"))

        # ============== stage A: GRU slice ==============
        wih_sb = big.tile([128, 3, 2 * H], F32, tag="big")
        whh_sb = big.tile([128, 3, H], F32, tag="big")
        nc.sync.dma_start(out=wih_sb, in_=t_wih.ap())
        nc.sync.dma_start(out=whh_sb, in_=t_whh.ap())

        x_rep = big.tile([128, 2 * H], F32, tag="big")
        h_rep = consts.tile([128, H], F32, tag="rep1024")
        nc.gpsimd.dma_start(out=x_rep, in_=t_xg.ap().unsqueeze(0).to_broadcast([128, 2 * H]))
        nc.gpsimd.dma_start(out=h_rep, in_=t_hvec.ap().unsqueeze(0).to_broadcast([128, H]))

        h_own = consts.tile([128, 1], F32)
        nc.gpsimd.dma_start(out=h_own, in_=t_hown.ap().rearrange("(p o) -> p o", o=1))
        bg_sb = consts.tile([128, 4], F32)
        nc.gpsimd.dma_start(out=bg_sb, in_=t_bg.ap())

        # encoder shard (prefetch now; used in stage B)
        enc_sb = big.tile([128, 8, H], F32, tag="big")
        nc.sync.dma_start(out=enc_sb, in_=t_enc.ap())
        # output-projection bias (also reused as the final output buffer)
        bout_sb = consts.tile([1, VKP], F32)
        nc.gpsimd.dma_start(out=bout_sb, in_=t_bout.ap().rearrange("(o f) -> o f", o=1))

        tmp = consts.tile([128, H], F32)
        g = consts.tile([128, 8], F32)  # i_r i_z i_n h_r h_z h_n, 2 scratch
        for j in range(3):
            # 2H-wide dot as two H-wide fused mul-reduces + add
            nc.vector.scalar_tensor_tensor(
                out=tmp, in0=wih_sb[:, j, 0:H], scalar=1.0, in1=x_rep[:, 0:H],
                op0=OP.mult, op1=OP.mult, accum_out=g[:, 6:7])
            nc.vector.scalar_tensor_tensor(
                out=tmp, in0=wih_sb[:, j, H:2 * H], scalar=1.0, in1=x_rep[:, H:2 * H],
                op0=OP.mult, op1=OP.mult, accum_out=g[:, 7:8])
            nc.vector.tensor_add(g[:, j:j + 1], g[:, 6:7], g[:, 7:8])
        for j in range(3):
            nc.vector.scalar_tensor_tensor(
                out=tmp, in0=whh_sb[:, j, :], scalar=1.0, in1=h_rep,
                op0=OP.mult, op1=OP.mult, accum_out=g[:, 3 + j:4 + j])

        pre_r = consts.tile([128, 1], F32)
        pre_z = consts.tile([128, 1], F32)
        r_t = consts.tile([128, 1], F32)
        z_t = consts.tile([128, 1], F32)
        rhn = consts.tile([128, 1], F32)
        pre_n = consts.tile([128, 1], F32)
        n_t = consts.tile([128, 1], F32)
        hmn = consts.tile([128, 1], F32)
        h_new = consts.tile([128, 1], F32)

        nc.vector.tensor_add(pre_r, g[:, 0:1], g[:, 3:4])
        nc.scalar.activation(out=r_t, in_=pre_r, func=AF.Sigmoid, bias=bg_sb[:, 0:1])
        nc.vector.tensor_add(pre_z, g[:, 1:2], g[:, 4:5])
        nc.scalar.activation(out=z_t, in_=pre_z, func=AF.Sigmoid, bias=bg_sb[:, 1:2])
        # rhn = (h_n_raw + b_hn) * r
        nc.vector.scalar_tensor_tensor(
            out=rhn, in0=g[:, 5:6], scalar=bg_sb[:, 3:4], in1=r_t, op0=OP.add, op1=OP.mult)
        # pre_n = (i_n_raw + b_in) + rhn
        nc.vector.scalar_tensor_tensor(
            out=pre_n, in0=g[:, 2:3], scalar=bg_sb[:, 2:3], in1=rhn, op0=OP.add, op1=OP.add)
        nc.scalar.activation(out=n_t, in_=pre_n, func=AF.Tanh)
        # h_new = (h - n) * z + n
        nc.vector.tensor_sub(hmn, h_own, n_t)
        nc.vector.scalar_tensor_tensor(
            out=h_new, in0=hmn, scalar=z_t, in1=n_t, op0=OP.mult, op1=OP.add)

        if stage == "A1":
            nc.gpsimd.dma_start(out=t_hout.ap()[0:HK], in_=h_new)
            raise _StageCut
        # AllGather #1: h_new slices -> full h_new
        ag1_in = dram.tile([HK], F32)
        ag1_out = dram.tile([H], F32)
        nc.gpsimd.dma_start(out=ag1_in[:], in_=h_new)
        nc.gpsimd.collective_compute(
            "AllGather", OP.bypass, replica_groups=rg,
            ins=[ag1_in.opt()], outs=[ag1_out.opt()])
        nc.gpsimd.dma_start(out=t_hout.ap(), in_=ag1_out[:])

        hn_rep = consts.tile([128, H], F32, tag="rep1024")
        nc.gpsimd.dma_start(out=hn_rep, in_=ag1_out.unsqueeze(0).to_broadcast([128, H]))

        hcat32 = consts.tile([128, 8], F32)
        hcat16 = consts.tile([128, 8], F16)
        nc.gpsimd.dma_start(out=hcat32, in_=ag1_out.rearrange("(c p) -> p c", c=8))
        nc.vector.tensor_copy(out=hcat16, in_=hcat32)

        if stage == "A2":
            raise _StageCut
        # ============== stage B: attention ==============
        sc = consts.tile([128, 8], F32)
        for c in range(8):
            nc.vector.scalar_tensor_tensor(
                out=tmp, in0=enc_sb[:, c, :], scalar=1.0, in1=hn_rep,
                op0=OP.mult, op1=OP.mult, accum_out=sc[:, c:c + 1])

        mloc = consts.tile([128, 1], F32)
        mflat = consts.tile([1, 128], F32)
        mk = consts.tile([1, 1], F32)
        nmk = consts.tile([1, 1], F32)
        negm = consts.tile([128, 1], F32)
        expv = consts.tile([128, 8], F32)
        zrow = consts.tile([128, 1], F32)
        zflat = consts.tile([1, 128], F32)
        zk = consts.tile([1, 1], F32)
        nc.vector.tensor_reduce(out=mloc, in_=sc, axis=AX, op=OP.max)
        nc.gpsimd.dma_start(out=mflat, in_=mloc)
        nc.vector.tensor_reduce(out=mk, in_=mflat, axis=AX, op=OP.max)
        nc.vector.tensor_scalar_mul(nmk, mk, -1.0)
        bb1 = dram.tile([1], F32)
        nc.gpsimd.dma_start(out=bb1[:], in_=nmk)
        nc.gpsimd.dma_start(out=negm, in_=bb1.unsqueeze(0).to_broadcast([128, 1]))
        nc.scalar.activation(out=expv, in_=sc, func=AF.Exp, bias=negm, accum_out=zrow)
        nc.gpsimd.dma_start(out=zflat, in_=zrow)
        nc.vector.tensor_reduce(out=zk, in_=zflat, axis=AX, op=OP.add)

        # context partial (unnormalized): ck = sum_s exp_s * enc_s  (f32 PE)
        ck_ps = psA.tile([1, H], F32)
        for c in range(8):
            for nb in range(2):
                nc.tensor.matmul(
                    ck_ps[0:1, nb * 512:(nb + 1) * 512],
                    lhsT=expv[:, c:c + 1],
                    rhs=enc_sb[:, c, nb * 512:(nb + 1) * 512],
                    start=(c == 0), stop=(c == 7))

        # AllGather #2: packet [ck(1024), m, z]
        PK = H + 8  # packet padded to a 32-byte multiple for the collective
        pack = consts.tile([1, PK], F32)
        nc.vector.memset(pack[0:1, H + 2:PK], 0.0)
        nc.scalar.copy(out=pack[0:1, 0:H], in_=ck_ps[0:1, :])
        nc.scalar.copy(out=pack[0:1, H:H + 1], in_=mk)
        nc.scalar.copy(out=pack[0:1, H + 1:H + 2], in_=zk)
        if stage == "B1":
            nc.gpsimd.dma_start(out=t_cout.ap(), in_=pack[0:1, 0:H])
            raise _StageCut
        ag2_in = dram.tile([PK], F32)
        ag2_out = dram.tile([NCORES * PK], F32)
        nc.gpsimd.dma_start(out=ag2_in[:], in_=pack)
        nc.gpsimd.collective_compute(
            "AllGather", OP.bypass, replica_groups=rg,
            ins=[ag2_in.opt()], outs=[ag2_out.opt()])

        stats = consts.tile([8, H], F32)
        nc.gpsimd.dma_start(out=stats, in_=ag2_out.rearrange("(r f) -> r f", r=8)[:, 0:H])
        m8 = consts.tile([1, 8], F32)
        z8 = consts.tile([1, 8], F32)
        nc.gpsimd.dma_start(out=m8, in_=ag2_out.rearrange("(r f) -> r f", r=8)[:, H:H + 1])
        nc.gpsimd.dma_start(out=z8, in_=ag2_out.rearrange("(r f) -> r f", r=8)[:, H + 1:H + 2])
        mgl = consts.tile([1, 1], F32)
        nmgl = consts.tile([1, 1], F32)
        scl8 = consts.tile([1, 8], F32)
        zz8 = consts.tile([1, 8], F32)
        zgl = consts.tile([1, 1], F32)
        rz = consts.tile([1, 1], F32)
        nc.vector.tensor_reduce(out=mgl, in_=m8, axis=AX, op=OP.max)
        nc.vector.tensor_scalar_mul(nmgl, mgl, -1.0)
        nc.scalar.activation(out=scl8, in_=m8, func=AF.Exp, bias=nmgl)
        nc.vector.tensor_mul(zz8, z8, scl8)
        nc.vector.tensor_reduce(out=zgl, in_=zz8, axis=AX, op=OP.add)
        nc.vector.reciprocal(rz, zgl)

        # context = sum_r (scl_r / Z) * ck_r   via weighted PE combine
        w8f = consts.tile([1, 8], F32)
        nc.vector.tensor_scalar_mul(w8f, scl8, rz)
        bb8 = dram.tile([8], F32)
        nc.gpsimd.dma_start(out=bb8[:], in_=w8f)
        w8 = consts.tile([8, 1], F32)
        nc.gpsimd.dma_start(out=w8, in_=bb8.rearrange("(p o) -> p o", o=1))
        ctx_ps = psA.tile([1, H], F32)
        for nb in range(2):
            nc.tensor.matmul(
                ctx_ps[0:1, nb * 512:(nb + 1) * 512],
                lhsT=w8, rhs=stats[:, nb * 512:(nb + 1) * 512],
                start=True, stop=True)
        ctx_sb = consts.tile([1, H], F32)
        nc.scalar.copy(out=ctx_sb, in_=ctx_ps[0:1, :])
        ctx_dram = dram.tile([H], F32)
        nc.gpsimd.dma_start(out=ctx_dram[:], in_=ctx_sb)
        nc.gpsimd.dma_start(out=t_cout.ap(), in_=ctx_dram[:])
        ccat32 = consts.tile([128, 8], F32)
        ccat16 = consts.tile([128, 8], F16)
        nc.gpsimd.dma_start(out=ccat32, in_=ctx_dram.rearrange("(c p) -> p c", c=8))
        nc.vector.tensor_copy(out=ccat16, in_=ccat32)

        # attention weights output: expv * exp(m_k - M) / Z
        dmk = consts.tile([1, 1], F32)
        e1 = consts.tile([1, 1], F32)
        so_s = consts.tile([1, 1], F32)
        so_bc = consts.tile([128, 1], F32)
        attn_sb = consts.tile([128, 8], F32)
        nc.vector.tensor_sub(dmk, mk, mgl)
        nc.scalar.activation(out=e1, in_=dmk, func=AF.Exp)
        nc.vector.tensor_mul(so_s, e1, rz)
        bbs = dram.tile([1], F32)
        nc.gpsimd.dma_start(out=bbs[:], in_=so_s)
        nc.gpsimd.dma_start(out=so_bc, in_=bbs.unsqueeze(0).to_broadcast([128, 1]))
        nc.vector.tensor_scalar_mul(attn_sb, expv, so_bc)
        nc.gpsimd.dma_start(out=t_aout.ap().rearrange("(c p) -> p c", c=8), in_=attn_sb)

        if stage == "B2":
            raise _StageCut
        # ============== stage C: output projection stream ==============
        # logits = bias + h_new-half + context-half, streamed in two passes so
        # the h_new half overlaps the attention stage.
        logits = consts.tile([1, VKP], F32)
        mx = consts.tile([1, NCH], F32)
        zc = consts.tile([1, NCH], F32)
        nmx = consts.tile([1, NCH], F32)
        exs = consts.tile([1, 512], F32)

        for half, lhs in ((0, hcat16), (1, ccat16)):
            for v0, vw in V_STRIPES:
                wts = []
                for kg in range(2):
                    kgg = half * 2 + kg
                    wt = big.tile([128, 4, 2192], F16, tag="big")
                    nc.sync.dma_start(
                        out=wt[:, :, 0:vw],
                        in_=t_wout.ap()[kgg * 4:(kgg + 1) * 4, :, v0:v0 + vw]
                            .rearrange("k p v -> p k v"))
                    wts.append(wt)
                nch_s = (vw + 511) // 512
                pss = [psB.tile([1, 512], F32, tag="ps", name=f"ps_{half}_{v0}_{i}")
                       for i in range(nch_s)]
                for kg in range(2):
                    for ch in range(nch_s):
                        cw = min(512, vw - ch * 512)
                        for kk in range(4):
                            kc = kg * 4 + kk
                            nc.tensor.matmul(
                                pss[ch][0:1, 0:cw],
                                lhsT=lhs[:, kc:kc + 1],
                                rhs=wts[kg][:, kk, ch * 512:ch * 512 + cw],
                                start=(kc % 8 == 0), stop=(kc % 8 == 7))
                for ch in range(nch_s):
                    cw = min(512, vw - ch * 512)
                    gc = (v0 + ch * 512) // 512
                    c0 = gc * 512
                    if half == 0:
                        nc.vector.tensor_add(
                            logits[0:1, c0:c0 + cw], pss[ch][0:1, 0:cw],
                            bout_sb[0:1, c0:c0 + cw])
                    else:
                        nc.vector.tensor_add(
                            bout_sb[0:1, c0:c0 + cw], pss[ch][0:1, 0:cw],
                            logits[0:1, c0:c0 + cw])
                        # incremental chunk stats once the chunk is final
                        nc.vector.tensor_reduce(
                            out=mx[0:1, gc:gc + 1], in_=bout_sb[0:1, c0:c0 + cw],
                            axis=AX, op=OP.max)
                        nc.vector.tensor_scalar_mul(
                            nmx[0:1, gc:gc + 1], mx[0:1, gc:gc + 1], -1.0)
                        nc.scalar.activation(
                            out=exs[0:1, 0:cw], in_=bout_sb[0:1, c0:c0 + cw],
                            func=AF.Exp, bias=nmx[0:1, gc:gc + 1],
                            accum_out=zc[0:1, gc:gc + 1])
        if stage == "C1":
            nc.sync.dma_start(out=t_out.ap()[0:VK], in_=bout_sb[0:1, 0:VK])
            raise _StageCut
        # bout_sb pad tail [VK:VKP] still holds bias = -1e30, exactly what the
        # padded output needs; stats above only ever saw real columns.

        # combine local chunk stats -> (m_loc, z_loc)
        mloc3 = consts.tile([1, 1], F32)
        nmloc3 = consts.tile([1, 1], F32)
        scl13 = consts.tile([1, NCH], F32)
        zscl = consts.tile([1, NCH], F32)
        zloc3 = consts.tile([1, 1], F32)
        nc.vector.tensor_reduce(out=mloc3, in_=mx, axis=AX, op=OP.max)
        nc.vector.tensor_scalar_mul(nmloc3, mloc3, -1.0)
        nc.scalar.activation(out=scl13, in_=mx, func=AF.Exp, bias=nmloc3)
        nc.vector.tensor_mul(zscl, zc, scl13)
        nc.vector.tensor_reduce(out=zloc3, in_=zscl, axis=AX, op=OP.add)

        # AllGather #3: (m_loc, z_loc)
        pk3 = consts.tile([1, 8], F32)
        nc.vector.memset(pk3[0:1, 2:8], 0.0)
        nc.scalar.copy(out=pk3[0:1, 0:1], in_=mloc3)
        nc.scalar.copy(out=pk3[0:1, 1:2], in_=zloc3)
        ag3_in = dram.tile([8], F32)
        ag3_out = dram.tile([64], F32)
        nc.gpsimd.dma_start(out=ag3_in[:], in_=pk3)
        nc.gpsimd.collective_compute(
            "AllGather", OP.bypass, replica_groups=rg,
            ins=[ag3_in.opt()], outs=[ag3_out.opt()])
        m83 = consts.tile([1, 8], F32)
        z83 = consts.tile([1, 8], F32)
        nc.gpsimd.dma_start(out=m83, in_=ag3_out.rearrange("(r f) -> r f", r=8)[:, 0:1])
        nc.gpsimd.dma_start(out=z83, in_=ag3_out.rearrange("(r f) -> r f", r=8)[:, 1:2])
        mg3 = consts.tile([1, 1], F32)
        nmg3 = consts.tile([1, 1], F32)
        sc83 = consts.tile([1, 8], F32)
        zz83 = consts.tile([1, 8], F32)
        zg3 = consts.tile([1, 1], F32)
        lnz = consts.tile([1, 1], F32)
        lse = consts.tile([1, 1], F32)
        nlse = consts.tile([1, 1], F32)
        nc.vector.tensor_reduce(out=mg3, in_=m83, axis=AX, op=OP.max)
        nc.vector.tensor_scalar_mul(nmg3, mg3, -1.0)
        nc.scalar.activation(out=sc83, in_=m83, func=AF.Exp, bias=nmg3)
        nc.vector.tensor_mul(zz83, z83, sc83)
        nc.vector.tensor_reduce(out=zg3, in_=zz83, axis=AX, op=OP.add)
        nc.scalar.activation(out=lnz, in_=zg3, func=AF.Ln)
        nc.vector.tensor_add(lse, mg3, lnz)
        nc.vector.tensor_scalar_mul(nlse, lse, -1.0)

        # out = final_logits - LSE; final logits live in bout_sb, result goes
        # into the dead logits buffer (split across ACT and DVE to halve the
        # serial tail)
        half_w = VKP // 2
        nc.scalar.activation(out=logits[0:1, 0:half_w], in_=bout_sb[0:1, 0:half_w],
                             func=AF.Identity, bias=nlse)
        nc.vector.tensor_scalar_add(logits[0:1, half_w:VKP], bout_sb[0:1, half_w:VKP], nlse)
        nc.sync.dma_start(out=t_out.ap(), in_=logits)

      except _StageCutT:
        pass
    nc.compile()
    return nc


def _prep_inputs(word_input, last_context, last_hidden, encoder_outputs,
                 embedding, W_ih, W_hh, b_ih, b_hh, W_out, b_out):
    f32 = np.float32
    word = int(np.asarray(word_input).reshape(-1)[0])
    emb = np.asarray(embedding, dtype=f32)[word]
    xg = np.concatenate([emb, np.asarray(last_context, dtype=f32)[0]]).astype(f32)
    h = np.asarray(last_hidden, dtype=f32)[0, 0]
    enc = np.asarray(encoder_outputs, dtype=f32)[:, 0, :]
    Wih = np.asarray(W_ih, dtype=f32)
    Whh = np.asarray(W_hh, dtype=f32)
    bih = np.asarray(b_ih, dtype=f32)
    bhh = np.asarray(b_hh, dtype=f32)
    Wout = np.asarray(W_out, dtype=f32)
    bout = np.asarray(b_out, dtype=f32)

    WoutP16 = np.zeros((VP, 2 * H), np.float16)
    WoutP16[:V] = Wout.astype(np.float16)
    boutP = np.full(VP, NEG, dtype=f32)
    boutP[:V] = bout

    in_maps = []
    for k in range(NCORES):
        sl = slice(k * HK, (k + 1) * HK)
        wih_k = np.ascontiguousarray(
            np.stack([Wih[0 * H:1 * H][sl], Wih[1 * H:2 * H][sl], Wih[2 * H:3 * H][sl]], axis=1))
        whh_k = np.ascontiguousarray(
            np.stack([Whh[0 * H:1 * H][sl], Whh[1 * H:2 * H][sl], Whh[2 * H:3 * H][sl]], axis=1))
        bg_k = np.ascontiguousarray(np.stack([
            bih[0 * H:1 * H][sl] + bhh[0 * H:1 * H][sl],
            bih[1 * H:2 * H][sl] + bhh[1 * H:2 * H][sl],
            bih[2 * H:3 * H][sl],
            bhh[2 * H:3 * H][sl]], axis=1))
        enc_k = np.ascontiguousarray(
            enc[k * SK:(k + 1) * SK].reshape(8, 128, H).transpose(1, 0, 2))
        wout_k = np.ascontiguousarray(
            WoutP16[k * VK:(k + 1) * VK].T.reshape(KC, 128, VK))
        bout_k = np.full(VKP, NEG, dtype=f32)
        bout_k[:VK] = boutP[k * VK:(k + 1) * VK]
        in_maps.append({
            "wih": wih_k, "whh": whh_k, "xg": xg, "hvec": h, "hown": h[sl].copy(),
            "bg": bg_k, "enc": enc_k, "wout": wout_k, "bout": bout_k,
        })
    return in_maps


def _assemble(outs):
    out_full = np.concatenate([outs[k]["out_ls"][:VK] for k in range(NCORES)])[:V]
    output = out_full.reshape(1, V).astype(np.float32)
    context = outs[0]["ctx_out"].reshape(1, H).astype(np.float32)
    h_new = outs[0]["h_out"].reshape(1, 1, H).astype(np.float32)
    attn = np.concatenate([outs[k]["attn_out"] for k in range(NCORES)]).reshape(1, 1, S).astype(np.float32)
    return output, context, h_new, attn


def kernel(**inputs):
    if "nc" not in _CACHE:
        _CACHE["nc"] = _build()
    nc = _CACHE["nc"]
    in_maps = _prep_inputs(**inputs)
    res = run_bass_kernel_spmd(nc, in_maps, core_ids=list(range(NCORES)),
                               **_CACHE.get("run_kwargs", {}))
    _CACHE["last_results"] = res
    return _assemble(res.results)
